# revision 17
# baseline (speedup 1.0000x reference)
"""CapsLayer2D Trainium2 kernel (8-core SPMD, data-parallel over batch).

Math: per position p (of B*R*C) and capsule n:
  U[n,i,o] = sum_e x[p,i,e] * W[n,i,e,o]          (u_hat)
  b0 = 1/64; 2x { v = squash(sum_i b*U); b += sum_o U*v }; out = squash(sum_i b*U)

Mapping:
  - 8 cores, 2 batches each -> 392 positions/core, 4 pos-blocks of 98.
  - Phase 1: S[p,n,o] = sum_{i,e} x*W as dense K=1024 accumulating matmuls
    (v0 = squash(S/64) since b0 is uniform).
  - Phase 2: per (block, n-pair) unit, u_hat materialized into PSUM via
    block-diagonal-W matmuls (stationary = xT chunk, moving = BD(W), N=256),
    then routing iterations as DVE mul + segmented-reduce ops reading PSUM.
  - Host pre-builds xT (transposed inputs), BD(W), dense W.
"""
import numpy as np

import concourse.bacc as bacc
import concourse.bass as bass
import concourse.mybir as mybir
import concourse.tile as tile
from concourse.bass_utils import run_bass_kernel_spmd

N_CORES = 8
B, R, C = 16, 14, 14
N_IN, D_IN = 64, 16          # i, e
N_CAPS, CAPS_DIM = 10, 16    # n, o
IE = N_IN * D_IN             # 1024
POS = (B // N_CORES) * R * C # 392 positions per core
BLK = 98                     # pos-block size
NBLK = POS // BLK            # 4
NF = N_CAPS // 2             # 5 units of 2 capsules
NCH = IE // 128              # 8 contraction chunks
F32 = mybir.dt.float32

# u_hat matmuls run in bf16 (1 col/cycle at any N; fp32 is 4x slower,
# fp32r needs producer-side rounding the DMA can't provide).
BF16 = mybir.dt.bfloat16

# of every 5 units, how many run their mul-passes on GpSimd
GPS_PER_5 = 2


def _squash(nc, pool, s_ap, v_ap, n):
    """v = squash(s): s_ap/v_ap are [98, n, 16] APs; n capsules."""
    P = s_ap.shape[0]
    sq = pool.tile([P, n * 16], F32, tag="sq")
    nc.scalar.activation(sq[:].rearrange("p (n o) -> p n o", o=16), s_ap,
                         mybir.ActivationFunctionType.Square)
    q = pool.tile([P, n], F32, tag="q")
    nc.vector.tensor_reduce(q[:], sq[:].rearrange("p (n o) -> p n o", o=16),
                            axis=mybir.AxisListType.X, op=mybir.AluOpType.add)
    rt = pool.tile([P, n], F32, tag="rt")
    nc.scalar.activation(rt[:], q[:], mybir.ActivationFunctionType.Sqrt)
    qp = pool.tile([P, n], F32, tag="qp")
    nc.vector.tensor_scalar_add(qp[:], q[:], 1.0)
    rc = pool.tile([P, n], F32, tag="rc")
    nc.vector.reciprocal(rc[:], qp[:])
    al = pool.tile([P, n], F32, tag="al")
    nc.vector.tensor_mul(al[:], rt[:], rc[:])
    alb = al[:].unsqueeze(2).broadcast_to([P, n, 16])
    nc.vector.tensor_mul(v_ap, s_ap, alb)


def build_kernel(dbg=False):
    nc = bacc.Bacc("TRN2", target_bir_lowering=False, debug=False,
                   num_devices=N_CORES)
    xT = nc.dram_tensor("xT", [IE, POS], F32, kind="ExternalInput").ap()
    bdw = nc.dram_tensor("bdw", [128, NCH * N_CAPS * 128], BF16,
                         kind="ExternalInput").ap()
    wd = nc.dram_tensor("wd", [IE, N_CAPS * 16], F32, kind="ExternalInput").ap()
    out = nc.dram_tensor("out", [POS, N_CAPS * 16], F32,
                         kind="ExternalOutput").ap()
    if dbg:
        dbg_s0 = nc.dram_tensor("dbg_s0", [BLK, NBLK * 160], F32,
                                kind="ExternalOutput").ap()
        dbg_v0 = nc.dram_tensor("dbg_v0", [BLK, NBLK * 160], F32,
                                kind="ExternalOutput").ap()
        dbg_u = nc.dram_tensor("dbg_u", [BLK, 2048], F32,
                               kind="ExternalOutput").ap()
        dbg_b1 = nc.dram_tensor("dbg_b1", [BLK, 128], F32,
                                kind="ExternalOutput").ap()

    with tile.TileContext(nc) as tc:
        with tc.tile_pool(name="const", bufs=1) as const, \
             tc.tile_pool(name="work", bufs=3) as work:
            bdw_t = const.tile([128, NCH * N_CAPS * 128], BF16)
            nc.sync.dma_start(bdw_t[:], bdw[:])
            xtb_t = const.tile([128, NCH * POS], BF16)   # bf16 xT for u_hat
            s0_t = const.tile([BLK, NBLK * 160], F32)    # S/64 per block
            v0_t = const.tile([BLK, NBLK * 160], F32)
            out_t = const.tile([BLK, NBLK * 160], F32)

            # ---- phase 1: S = sum_ie x*W ; v0 = squash(S/64) ----
            with tc.tile_pool(name="p1", bufs=1) as p1, \
                 tc.tile_pool(name="psum_s", bufs=4, space="PSUM") as psum_s:
                xt_t = p1.tile([128, NCH * POS], F32)    # chunk g at g*POS
                for g in range(NCH):
                    nc.sync.dma_start(xt_t[:, g * POS:(g + 1) * POS],
                                      xT[g * 128:(g + 1) * 128, :])
                wd_t = p1.tile([128, NCH * N_CAPS * 16], F32)
                for g in range(NCH):
                    nc.sync.dma_start(wd_t[:, g * 160:(g + 1) * 160],
                                      wd[g * 128:(g + 1) * 128, :])
                nc.vector.tensor_copy(xtb_t[:], xt_t[:])
                for b in range(NBLK):
                    for f in range(NF):
                        ps = psum_s.tile([BLK, 32], F32, tag="ps")
                        for g in range(NCH):
                            nc.tensor.matmul(
                                ps[:],
                                xt_t[:, g * POS + b * BLK: g * POS + (b + 1) * BLK],
                                wd_t[:, g * 160 + f * 32: g * 160 + (f + 1) * 32],
                                start=(g == 0), stop=(g == NCH - 1))
                        nc.scalar.activation(
                            s0_t[:, b * 160 + f * 32: b * 160 + (f + 1) * 32],
                            ps[:], mybir.ActivationFunctionType.Copy,
                            scale=1.0 / N_IN)
                for b in range(NBLK):
                    sb = s0_t[:, b * 160:(b + 1) * 160].rearrange(
                        "p (n o) -> p n o", o=16)
                    vb = v0_t[:, b * 160:(b + 1) * 160].rearrange(
                        "p (n o) -> p n o", o=16)
                    _squash(nc, work, sb, vb, N_CAPS)

            # ---- phase 2: u_hat + 2 routing iterations, batched per block --
            # Unit (b,f) u_hat -> PSUM [p,(gi,n2,o)] (gi=8g+i8=i), ACT-drains
            # to bf16 SBUF ub[f*2048:]. Routing per block (5 units at once):
            #   it0: b1 = 1/64 + sum_o U*v0 ; v1 = squash(sum_i b1*U)
            #   it1: b2 = b1 + sum_o U*v1  ; out = squash(sum_i b2*U)
            # Products are bf16 DVE muls (2x mode); contractions are halving
            # add-trees (bf16 2x on wide levels, f32 tail) - ~2x faster than
            # tensor_reduce which has no 2x mode.
            with tc.tile_pool(name="ubp", bufs=2) as ubp, \
                 tc.tile_pool(name="big", bufs=1) as big, \
                 tc.tile_pool(name="psum_u", bufs=2, space="PSUM") as psum_u:
                for b in range(NBLK):
                    ub = ubp.tile([BLK, NF * 2048], BF16, tag="ub")
                    for f in range(NF):
                        up = psum_u.tile([BLK, 2048], F32, tag="up")
                        for g in range(NCH):
                            lhs = xtb_t[:, g * POS + b * BLK: g * POS + (b + 1) * BLK]
                            rhs = bdw_t[:, g * 1280:(g + 1) * 1280] \
                                .rearrange("p (i c) -> p i c", c=160) \
                                [:, :, f * 32:(f + 1) * 32]
                            nc.tensor.matmul(
                                up[:, g * 256:(g + 1) * 256], lhs, rhs,
                                start=True, stop=True)
                        nc.scalar.activation(ub[:, f * 2048:(f + 1) * 2048],
                                             up[:],
                                             mybir.ActivationFunctionType.Copy)
                        if dbg and b == 0 and f == 0:
                            ucp = work.tile([BLK, 2048], F32, tag="ucp")
                            nc.vector.tensor_copy(ucp[:], up[:])
                            nc.sync.dma_start(dbg_u[:], ucp[:])

                    bco = work.tile([BLK, NF * 128], F32, tag="bco")  # (f,gi,n2)
                    nc.vector.memset(bco[:], 1.0 / N_IN)
                    vb16 = work.tile([BLK, 160], BF16, tag="vb16")
                    nc.vector.tensor_copy(vb16[:],
                                          v0_t[:, b * 160:(b + 1) * 160])
                    Ub = ub[:].rearrange("p (f gi no) -> p f gi no",
                                         f=NF, gi=64, no=32)
                    for it in range(2):
                        # agreement: bco += sum_o U*v (tree over o=16)
                        P = big.tile([BLK, NF * 2048], BF16, tag="P")
                        vbb = vb16[:].rearrange("p (f no) -> p f no", no=32) \
                            .unsqueeze(2).broadcast_to([BLK, NF, 64, 32])
                        nc.vector.tensor_mul(
                            P[:].rearrange("p (f gi no) -> p f gi no",
                                           f=NF, gi=64, no=32), Ub, vbb)
                        with nc.allow_low_precision("bf16 tree sums"):
                            Pv = P[:].rearrange("p (s o) -> p s o", o=16)
                            t1 = big.tile([BLK, NF * 1024], BF16, tag="t1")
                            t1v = t1[:].rearrange("p (s o) -> p s o", o=8)
                            nc.vector.tensor_add(t1v, Pv[:, :, 0:8], Pv[:, :, 8:16])
                            t2 = big.tile([BLK, NF * 512], BF16, tag="t2")
                            t2v = t2[:].rearrange("p (s o) -> p s o", o=4)
                            nc.vector.tensor_add(t2v, t1v[:, :, 0:4], t1v[:, :, 4:8])
                            t3 = big.tile([BLK, NF * 256], BF16, tag="t3")
                            t3v = t3[:].rearrange("p (s o) -> p s o", o=2)
                            nc.vector.tensor_add(t3v, t2v[:, :, 0:2], t2v[:, :, 2:4])
                            agr = work.tile([BLK, NF * 128], F32, tag="agr")
                            nc.vector.tensor_add(
                                agr[:].rearrange("p (s o) -> p s o", o=1),
                                t3v[:, :, 0:1], t3v[:, :, 1:2])
                        nc.vector.tensor_add(bco[:], bco[:], agr[:])
                        if dbg and b == 0 and it == 0:
                            nc.sync.dma_start(dbg_b1[:], bco[:, 0:128])
                        # v-sum: s = sum_gi b*U (Q in (f,n,o,gi); tree over gi)
                        bcb = work.tile([BLK, NF * 128], BF16, tag="bcb")
                        nc.vector.tensor_copy(bcb[:], bco[:])
                        Q = big.tile([BLK, NF * 2048], BF16, tag="Q")
                        for f in range(NF):
                            Uq = ub[:, f * 2048:(f + 1) * 2048].rearrange(
                                "p (gi n o) -> p n o gi", gi=64, n=2, o=16)
                            bbf = bcb[:, f * 128:(f + 1) * 128].rearrange(
                                "p (gi n) -> p n gi", n=2) \
                                .unsqueeze(2).broadcast_to([BLK, 2, 16, 64])
                            nc.vector.tensor_mul(
                                Q[:, f * 2048:(f + 1) * 2048].rearrange(
                                    "p (n o gi) -> p n o gi", n=2, o=16),
                                Uq, bbf)
                        with nc.allow_low_precision("bf16 tree sums"):
                            Qv = Q[:].rearrange("p (s g) -> p s g", g=64)
                            q1 = big.tile([BLK, NF * 1024], BF16, tag="q1")
                            q1v = q1[:].rearrange("p (s g) -> p s g", g=32)
                            nc.vector.tensor_add(q1v, Qv[:, :, 0:32], Qv[:, :, 32:64])
                            q2 = big.tile([BLK, NF * 512], BF16, tag="q2")
                            q2v = q2[:].rearrange("p (s g) -> p s g", g=16)
                            nc.vector.tensor_add(q2v, q1v[:, :, 0:16], q1v[:, :, 16:32])
                            q3 = big.tile([BLK, NF * 256], BF16, tag="q3")
                            q3v = q3[:].rearrange("p (s g) -> p s g", g=8)
                            nc.vector.tensor_add(q3v, q2v[:, :, 0:8], q2v[:, :, 8:16])
                            q4 = big.tile([BLK, NF * 128], BF16, tag="q4")
                            q4v = q4[:].rearrange("p (s g) -> p s g", g=4)
                            nc.vector.tensor_add(q4v, q3v[:, :, 0:4], q3v[:, :, 4:8])
                            q5 = big.tile([BLK, NF * 64], BF16, tag="q5")
                            q5v = q5[:].rearrange("p (s g) -> p s g", g=2)
                            nc.vector.tensor_add(q5v, q4v[:, :, 0:2], q4v[:, :, 2:4])
                            s_blk = work.tile([BLK, 160], F32, tag="s_blk")
                            nc.vector.tensor_add(
                                s_blk[:].rearrange("p (s g) -> p s g", g=1),
                                q5v[:, :, 0:1], q5v[:, :, 1:2])
                        # squash (batched over the block's 10 capsules)
                        if it == 0:
                            v_blk = work.tile([BLK, 160], F32, tag="v_blk")
                            _squash(nc, work,
                                    s_blk[:].rearrange("p (n o) -> p n o", o=16),
                                    v_blk[:].rearrange("p (n o) -> p n o", o=16),
                                    N_CAPS)
                            nc.vector.tensor_copy(vb16[:], v_blk[:])
                        else:
                            dst = out_t[:, b * 160:(b + 1) * 160]
                            _squash(nc, work,
                                    s_blk[:].rearrange("p (n o) -> p n o", o=16),
                                    dst.rearrange("p (n o) -> p n o", o=16),
                                    N_CAPS)

            for b in range(NBLK):
                nc.sync.dma_start(out[b * BLK:(b + 1) * BLK, :],
                                  out_t[:, b * 160:(b + 1) * 160])
            if dbg:
                nc.sync.dma_start(dbg_s0[:], s0_t[:])
                nc.sync.dma_start(dbg_v0[:], v0_t[:])
    nc.compile()
    return nc


def _host_prep(inputs, W):
    """Build per-core input maps from full inputs."""
    x = np.ascontiguousarray(inputs, dtype=np.float32).reshape(B, R * C, IE)
    Wf = np.ascontiguousarray(W, dtype=np.float32)  # [n, i, e, o]
    # bdw[(i8,e), (g,n,i8,o)]
    Wg = Wf.reshape(N_CAPS, 8, 8, D_IN, CAPS_DIM)   # [n, g, i8, e, o]
    bdw6 = np.zeros((8, D_IN, 8, 8, N_CAPS, CAPS_DIM), dtype=np.float32)
    for i8 in range(8):
        # [n, g, e, o] -> [e, g, n, o]
        bdw6[i8, :, :, i8, :, :] = Wg[:, :, i8, :, :].transpose(2, 1, 0, 3)
    import ml_dtypes
    bdw = bdw6.reshape(128, NCH * N_CAPS * 128).astype(ml_dtypes.bfloat16)
    wd = Wf.transpose(1, 2, 0, 3).reshape(IE, N_CAPS * CAPS_DIM)
    bpc = B // N_CORES
    in_maps = []
    for c in range(N_CORES):
        xc = x[c * bpc:(c + 1) * bpc].reshape(POS, IE)
        in_maps.append({
            "xT": np.ascontiguousarray(xc.T),
            "bdw": bdw,
            "wd": wd,
        })
    return in_maps


_NC_CACHE = []


def kernel(inputs: np.ndarray, W: np.ndarray) -> np.ndarray:
    in_maps = _host_prep(inputs, W)
    if not _NC_CACHE:
        _NC_CACHE.append(build_kernel())
    nc = _NC_CACHE[0]
    res = run_bass_kernel_spmd(nc, in_maps, list(range(N_CORES)))
    outs = [res.results[c]["out"] for c in range(N_CORES)]
    full = np.concatenate(outs, axis=0)  # [3136, 160]
    return full.reshape(B, R, C, N_CAPS, CAPS_DIM)
